# revision 53
# baseline (speedup 1.0000x reference)
"""ContextualConv1d Trainium2 kernel (polyphase scheme, v2).

Problem: grouped conv1d (N=32, C_in=256, L=4096, C_out=256, K=9, groups=4,
pad=4) + broadcast context term c @ c_weight.T + bias.

Sharding: data-parallel over batch N across 8 cores (4 batches/core).

Conv strategy — polyphase decomposition for full PE utilization:
  x is split host-side into even/odd phases. For one group, the matmul
  contraction packs (64 channels x 2 input phases) = 128 rows, and the
  output partitions pack (64 out channels x 2 output parities) = 128.
  The K=9 conv then becomes 5 accumulating matmuls (phase shifts s=0..4)
  with dense 128x128 stationary operands (~90% MAC utilization):

    ps[(o,d), m] += lhsT_s[(i,ph), (o,d)] * x2[(i,ph), m+s]
    lhsT_s[(i,0),(o,0)] = W[o,i,2s]    lhsT_s[(i,1),(o,0)] = W[o,i,2s+1]
    lhsT_s[(i,0),(o,1)] = W[o,i,2s-1]  lhsT_s[(i,1),(o,1)] = W[o,i,2s]
    (out-of-range taps are zero blocks)

  y[o, 2m+d] = ps[(o,d), m]. Phase split / parity merge are free host-side
  reshapes during shard/unshard.

v2 design (each point verified in TimelineSim and by paired repeat-loop
HW timing; single-shot cost model 88.5us -> 79.1us, HW p30 ~95us -> low
70s us/iter in quiet windows):
  - Queue discipline: SP ring carries ONLY loads (a load never waits on
    data, so prefetch — including the next repeat iteration's — is never
    parked behind a store's sem wait); ACT ring carries only store
    issues; ALL PSUM->SBUF drains run on DVE. With drains off the ACT
    ring, a late store under bus contention can no longer stall the
    drain stream and hold PSUM banks (HW: dve drains beat DVE/ACT-split
    drains in every paired window).
  - Ldweights dedup (_dedup_ldweights): the PE array keeps a loaded
    stationary until the next LDWEIGHTS, and the sgt loop issues 4
    matmuls per stationary, so 3 of 4 emitted InstLdweights are dropped
    post-schedule (326 -> ~129). Verified bit-identical on HW; ~4us/iter
    faster (HW LDWEIGHTS costs real time that the cost model ignores).
  - Startup: bootstrap DMA order x(0,0) -> wq[g0,s0] (32KB: unblocks the
    first matmul) -> packed cc -> rest of wq -> x(0,1..3). The ctx
    matmuls are emitted after the first conv row's matmuls in
    single-shot mode (ctx2 is only needed by the first drain), so conv
    is not queued behind ctx's wait for cc on the in-order PE queue.
    First conv MM at ~4.5us vs ~10us before.
  - PE p-state warm-up: WARM_MMS garbage matmuls on zeroed operands into
    a never-read PSUM range bridge the idle window until x(0,0) lands;
    without them the cost model holds the first ~19 conv matmuls at the
    LOW/MID p-state (788/427ns vs 213ns).
  - Small consts (cwT2 fp16 + cT) packed into one cc tensor = one DMA
    (the HWDGE serializer costs ~625ns per DMA instruction regardless of
    size); f32 bias rides a separate tiny tensor (tensor_scalar_add
    requires an f32 scalar operand).
  - DRAM x/out layouts are [NB, 128, G*len] so one DMA instruction can
    carry 1, 2 or 4 groups of a batch (contiguous per partition).
    GMERGE=2 halves the x-load/out-store DMA instruction count; batch-0
    loads stay per-group so the first matmul isn't gated on a merged
    transfer; last-batch stores stay per-group, and the last row drains/
    stores in decreasing-width chunks (1024/512/512 cols) so the post-
    final-MM tail is one [128,512] drain + one 128KB store.

Precision choices:
  - Conv inputs fp16: half the x DMA bytes and >= f32r PE speed.
    fp8 is ruled out by accuracy: e4m3 operand quantization alone costs
    ~2.5e-2 max rel err vs the 2e-2 gate.
  - Output stored fp16, upcast on host. End-to-end max rel err ~5.2e-4.
Roofline: PE fp16 floor ~68us/core steady state (320 x [128x128]@[128,512]
MMs at 213ns); DMA ~49us/core aggregate on the shared bus.
"""

import sys

if "/opt/trn_rl_repo" not in sys.path:
    sys.path.insert(0, "/opt/trn_rl_repo")

import numpy as np

N, C_IN, L = 32, 256, 4096
C_OUT, K, GROUPS = 256, 9, 4
C_DIM, PAD = 128, 4
NCORES = 8
NB = N // NCORES          # batches per core
M = L // 2                # output phase positions (2048)
MPAD = (L + 2 * PAD) // 2  # padded phase length (2052)
MT = 512                  # phase cols per PSUM tile (one bank of fp32)
NMT = M // MT             # 4 L-tiles per (n, g)
NSHIFT = 5                # phase shifts (= ceil(K/2))
CCW = GROUPS * 128 + NB + GROUPS  # packed consts width (cwT2 | cT | biasT2)

# Extra kwargs for run_bass_kernel_spmd (e.g. trace=True) set by a harness;
# the BassKernelResults lands in LAST_RESULT.
RUN_KWARGS: dict = {}
LAST_RESULT = None

_prog_cache: dict = {}

# Matmul input dtype for the conv path: "f32r" or "fp16" (2x less x DMA).
CONV_DTYPE = "fp16"
# Store the output phase tensor as fp16 (halves out-DMA; host upcasts).
OUT_FP16 = True
# PSUM->SBUF drain engines: "dve" (all drains on DVE; ACT carries only
# store issues, so a store's sem wait can never park the drain stream and
# hold PSUM banks under bus contention) or "split" (alternate DVE / ACT).
# Paired HW timing: dve beat split in every window (e.g. 70.7 vs 72.5 us
# p30 quiet, 92.7 vs 98.9 us contended).
DRAIN = "dve"
# Groups per x-load / out-store DMA for middle batches (1, 2, or 4).
GMERGE = 2
# x prefetch depth, in merged tiles (each carries GMERGE groups).
XBUFS = 6
# output staging tiles (each carries GMERGE groups of one batch).
OBUFS = 4
# PSUM tiles per drain op: 2 = [128,1024] two-bank drains.
DRAIN_WIDTH = 2
# Moving columns per matmul, in MT tiles: 1 = [128,512] (one PSUM bank per
# MM), 2 = [128,1024] (one MM per drain tile per shift; halves MM count).
MMW = 1
# Tail handling for the last row: "chunk" = sub-chunk drains + chunked
# stores (sgt order kept), "plain" = same as other rows.
TAIL = "chunk"
# PE p-state warm-up matmuls before the first conv MM (the cost model
# halves matmul speed until the PE has run continuously for ~3us; a cold
# first row otherwise runs at the LOW/MID p-state).
WARM_MMS = 31
# Remove back-to-back InstLdweights with an identical stationary AP: the PE
# array keeps the loaded weights until the next LDWEIGHTS, and the sgt loop
# issues 4 matmuls per stationary, so 3 of every 4 loads are redundant.
DEDUP_LDW = True


def _dedup_ldweights(nc, mybir):
    """Drop an InstLdweights when the previous PE-array load in the same
    block had the same stationary AP / perf mode / transpose flag. Waits on
    a dropped load move to the next PE instruction (its paired matmul);
    compile()'s generate_event_semaphores re-legalizes wait counts."""
    removed = 0
    for blk in nc.main_func.blocks:
        insts = list(blk.instructions)
        last_sig = None
        drop = []
        for idx, inst in enumerate(insts):
            if isinstance(inst, mybir.InstLdweights):
                sig = (str(inst.ins[0]), str(inst.perf_mode),
                       str(inst.is_transpose))
                if sig == last_sig:
                    si = inst.sync_info
                    if si is not None and (len(si.on_wait) or len(si.on_update)):
                        nxt = None
                        for j in range(idx + 1, len(insts)):
                            if insts[j].engine == mybir.EngineType.PE:
                                nxt = insts[j]
                                break
                        assert nxt is not None
                        nsi = nxt.sync_info
                        if nsi is None:
                            nxt.sync_info = si
                        else:
                            for w in si.on_wait:
                                nsi.on_wait.append(w)
                            for u in si.on_update:
                                nsi.on_update.append(u)
                    drop.append(idx)
                    removed += 1
                else:
                    last_sig = sig
            elif inst.engine == mybir.EngineType.PE and not isinstance(
                    inst, (mybir.InstMatmult, mybir.InstEventSemaphore,
                           mybir.InstDrain)):
                # Unknown PE-array effect: don't dedup across it.
                last_sig = None
        for idx in reversed(drop):
            del blk.instructions[idx]
    return removed


def _build_program(repeats: int = 1, conv_dtype: str | None = None,
                   out_fp16: bool | None = None, drain: str | None = None,
                   gmerge: int | None = None, xbufs: int | None = None,
                   drain_width: int | None = None, tail: str | None = None,
                   warm_mms: int | None = None, dedup_ldw: bool | None = None,
                   obufs: int | None = None, mmw: int | None = None,
                   unroll: int = 1):
    import concourse.bacc as bacc
    import concourse.mybir as mybir
    import concourse.tile as tile

    f32 = mybir.dt.float32
    f32r = {
        "f32r": mybir.dt.float32r,
        "fp16": mybir.dt.float16,
        "bf16": mybir.dt.bfloat16,
    }[conv_dtype or CONV_DTYPE]
    if out_fp16 is None:
        out_fp16 = OUT_FP16
    fout = mybir.dt.float16 if out_fp16 else f32
    if drain is None:
        drain = DRAIN
    if gmerge is None:
        gmerge = GMERGE
    if xbufs is None:
        xbufs = XBUFS
    if drain_width is None:
        drain_width = DRAIN_WIDTH
    if tail is None:
        tail = TAIL
    if warm_mms is None:
        warm_mms = WARM_MMS
    if dedup_ldw is None:
        dedup_ldw = DEDUP_LDW
    if obufs is None:
        obufs = OBUFS
    if mmw is None:
        mmw = MMW
    dw = drain_width
    assert NMT % dw == 0
    assert GROUPS % gmerge == 0
    assert mmw in (1, dw)

    nc = bacc.Bacc(None, target_bir_lowering=False, name="ctxconv1d")

    xq_d = nc.dram_tensor("xq", [NB, 128, GROUPS, MPAD], f32r,
                          kind="ExternalInput")
    wq_d = nc.dram_tensor("wq", [128, GROUPS, NSHIFT, 128], f32r,
                          kind="ExternalInput")
    cc_d = nc.dram_tensor("cc", [C_DIM, CCW], f32r, kind="ExternalInput")
    bias2_d = nc.dram_tensor("bias2", [128, GROUPS], f32, kind="ExternalInput")
    outq_d = nc.dram_tensor("outq", [NB, 128, GROUPS, M], fout,
                            kind="ExternalOutput")

    with tile.TileContext(nc) as tc:
        with (
            tc.tile_pool(name="consts", bufs=1) as consts,
            tc.tile_pool(name="xpool", bufs=xbufs) as xpool,
            tc.tile_pool(name="opool", bufs=obufs) as opool,
            # 7 of 8 PSUM banks for conv tiles; 1 bank (pcpool) for the
            # warm-up + ctx matmuls so they never gate conv tile rotation.
            tc.tile_pool(name="ppool", bufs=7 // dw, space="PSUM") as ppool,
            tc.tile_pool(name="pcpool", bufs=1, space="PSUM") as pcpool,
        ):
            wq_sb = consts.tile([128, GROUPS, NSHIFT, 128], f32r)
            cc_sb = consts.tile([C_DIM, CCW], f32r)
            bias2_sb = consts.tile([128, GROUPS], f32)
            ctx2_sb = consts.tile([128, GROUPS, NB], f32)
            warm_sb = consts.tile([128, 256], f32r)
            psc = pcpool.tile([128, MT], f32, tag="psc")

            ld_eng = nc.sync     # SP ring: all loads (never blocks on data)
            st_eng = nc.scalar   # ACT ring: all store issues

            # Bootstrap DMA order on the SP ring (HWDGE serializes issues at
            # ~625ns each; the shared DMA bus runs one transfer at a time):
            # x(0,0) first (longest pole), then the first conv MM's weight
            # slice (g0 s0), the packed small consts, x(0,1), the rest of the
            # weights, x(0,2..3). Only for the straight-line program — the
            # repeat loop issues per-iteration x loads inside body().
            single = repeats == 1 and unroll == 1
            x0_tiles = None
            if single:
                x0_tiles = [
                    xpool.tile([128, MPAD], f32r, name=f"x0g{g}", tag="x1")
                    for g in range(GROUPS)
                ]
                ld_eng.dma_start(x0_tiles[0][:], xq_d[0, :, 0, :])
                ld_eng.dma_start(wq_sb[:, 0, 0], wq_d[:, 0, 0])
                ld_eng.dma_start(cc_sb[:], cc_d[:])
                ld_eng.dma_start(wq_sb[:, 0, 1:NSHIFT], wq_d[:, 0, 1:NSHIFT])
                ld_eng.dma_start(bias2_sb[:], bias2_d[:])
                ld_eng.dma_start(x0_tiles[1][:], xq_d[0, :, 1, :])
                ld_eng.dma_start(wq_sb[:, 1:GROUPS], wq_d[:, 1:GROUPS])
                ld_eng.dma_start(x0_tiles[2][:], xq_d[0, :, 2, :])
                ld_eng.dma_start(x0_tiles[3][:], xq_d[0, :, 3, :])
            else:
                ld_eng.dma_start(wq_sb[:, 0], wq_d[:, 0])
                ld_eng.dma_start(cc_sb[:], cc_d[:])
                ld_eng.dma_start(bias2_sb[:], bias2_d[:])
                ld_eng.dma_start(wq_sb[:, 1:GROUPS], wq_d[:, 1:GROUPS])

            # PE p-state warm-up: garbage matmuls (zeroed operands) into a
            # never-read PSUM range keep the PE continuously busy from ~1us
            # until the first conv matmul's inputs land, so conv starts at
            # the MAX p-state instead of LOW/MID.
            if warm_mms:
                nc.vector.memset(warm_sb[:], 0.0)
                for _ in range(warm_mms):
                    nc.tensor.matmul(
                        psc[:, 256:384], warm_sb[:, 0:128],
                        warm_sb[:, 128:256], start=True, stop=True,
                    )

            cb = GROUPS * 128  # cc column offsets: cwT2 | cT | biasT2

            # ctx2[(o,d), g, n] = sum_dim c_weight[g*64+o, dim] * c[n, dim]
            # + bias (columns duplicated across the two output parities d).
            # In single-shot mode this is emitted after the first conv row's
            # matmuls (body calls it) so conv isn't queued behind ctx's wait
            # for the cc load; ctx2 is only needed by the first drain.
            def emit_ctx():
                for g in range(GROUPS):
                    nc.tensor.matmul(
                        psc[:, g * NB:(g + 1) * NB],
                        cc_sb[:, g * 128:(g + 1) * 128],
                        cc_sb[:, cb:cb + NB],
                        start=True, stop=True,
                    )
                for g in range(GROUPS):
                    nc.vector.tensor_scalar_add(
                        ctx2_sb[:, g, :], psc[:, g * NB:(g + 1) * NB],
                        bias2_sb[:, g:g + 1],
                    )

            if not single:
                emit_ctx()

            def drain_op(k, out_ap, ps_ap, bias_ap, eng=None):
                use_act = (drain == "split" and k % 2 == 1) if eng is None \
                    else eng == "act"
                if use_act:
                    nc.scalar.activation(
                        out_ap, ps_ap,
                        mybir.ActivationFunctionType.Identity,
                        bias=bias_ap,
                    )
                else:
                    nc.vector.tensor_scalar_add(out_ap, ps_ap, bias_ap)

            state = {"boot": x0_tiles, "ctx_pending": single}

            def body():
                dk = 0
                for n in range(NB):
                    # x tiles for this batch: batch 0 loads per group (so
                    # the first MMs aren't gated on a merged transfer),
                    # later batches load GMERGE groups per DMA. xap holds
                    # (tile, column base) so matmuls slice the tile once.
                    if n == 0:
                        if state["boot"] is not None:
                            xt = state["boot"]
                            state["boot"] = None
                        else:
                            xt = [xpool.tile([128, MPAD], f32r,
                                             name=f"x{n}g{g}", tag="x1")
                                  for g in range(GROUPS)]
                            for g in range(GROUPS):
                                ld_eng.dma_start(xt[g][:], xq_d[n, :, g, :])
                        xap = [(xt[g], 0) for g in range(GROUPS)]
                    else:
                        xap = []
                        for gp in range(GROUPS // gmerge):
                            x_t = xpool.tile([128, gmerge * MPAD], f32r,
                                             name=f"x{n}p{gp}", tag="xm")
                            ld_eng.dma_start(
                                x_t[:],
                                xq_d[n, :, gp * gmerge:(gp + 1) * gmerge, :],
                            )
                            for j in range(gmerge):
                                xap.append((x_t, j * MPAD))

                    last_n = n == NB - 1
                    for gp in range(GROUPS // gmerge):
                        if last_n:
                            o_ts = [opool.tile([128, M], fout,
                                               name=f"o{n}g{gp * gmerge + j}",
                                               tag="o1")
                                    for j in range(gmerge)]
                        else:
                            o_tm = opool.tile([128, gmerge * M], fout,
                                              name=f"o{n}p{gp}", tag="om")
                        for j in range(gmerge):
                            g = gp * gmerge + j
                            x_t, xb = xap[g]
                            if last_n:
                                o_t, ob = o_ts[j], 0
                            else:
                                o_t, ob = o_tm, j * M
                            last_row = last_n and g == GROUPS - 1
                            pss = [
                                ppool.tile([128, dw * MT], f32,
                                           name=f"ps{n}_{g}_{i}", tag="ps")
                                for i in range(NMT // dw)
                            ]
                            for s in range(NSHIFT):
                                for t in range(0, NMT, mmw):
                                    off = (t % dw) * MT
                                    c0 = xb + t * MT + s
                                    nc.tensor.matmul(
                                        pss[t // dw][:, off:off + mmw * MT],
                                        wq_sb[:, g, s, :],
                                        x_t[:, c0:c0 + mmw * MT],
                                        start=(s == 0),
                                        stop=(s == NSHIFT - 1),
                                    )
                            if single and n == 0 and g == 0 \
                                    and state["ctx_pending"]:
                                state["ctx_pending"] = False
                                emit_ctx()
                            if last_row and tail == "chunk":
                                # Tail: decreasing-width chunks (1024, 512,
                                # 512 cols) so the work left after the final
                                # MM is one small [128,512] drain + one
                                # 128KB store. First chunk drains on ACT
                                # (its data is ready a few MMs early and
                                # ACT is otherwise store-only), the rest on
                                # DVE; all stores on ACT. SP stays
                                # loads-only so the repeat loop's next-
                                # iteration prefetch is never parked behind
                                # a store's sem wait.
                                chunks = [(0, dw * MT)] + [
                                    (dw * MT + h * MT, MT) for h in range(dw)
                                ] if dw > 1 else [
                                    (h * MT, MT) for h in range(NMT)
                                ]
                                for ci, (c0, wd) in enumerate(chunks):
                                    i = c0 // (dw * MT)
                                    po = c0 - i * dw * MT
                                    drain_op(
                                        0, o_t[:, ob + c0:ob + c0 + wd],
                                        pss[i][:, po:po + wd],
                                        ctx2_sb[:, g, n:n + 1],
                                        eng="act" if ci == 0 else "dve",
                                    )
                                    st_eng.dma_start(
                                        outq_d[n, :, g, c0:c0 + wd],
                                        o_t[:, ob + c0:ob + c0 + wd],
                                    )
                                dk += len(chunks)
                            else:
                                for i in range(NMT // dw):
                                    c0 = ob + i * dw * MT
                                    drain_op(
                                        dk, o_t[:, c0:c0 + dw * MT],
                                        pss[i][:], ctx2_sb[:, g, n:n + 1],
                                    )
                                    dk += 1
                                if last_n:
                                    # Last batch: 2-chunk stores so the
                                    # final bus/issue work isn't one big
                                    # 512KB transfer bunched at the end.
                                    half = M // 2
                                    for h in range(2):
                                        st_eng.dma_start(
                                            outq_d[n, :, g,
                                                   h * half:(h + 1) * half],
                                            o_ts[j][:, h * half:(h + 1) * half],
                                        )
                        if not last_n:
                            st_eng.dma_start(
                                outq_d[n, :, gp * gmerge:(gp + 1) * gmerge, :],
                                o_tm[:],
                            )

            if repeats == 1:
                for _ in range(unroll):
                    body()
            else:
                # Big body (>256 insts/engine): arm back-edge prefetch so
                # repeat-loop timing isn't polluted by IRAM refetch stalls.
                with tc.For_i(
                    0, repeats, 1,
                    hint_engines=(
                        mybir.EngineType.PE,
                        mybir.EngineType.SP,
                        mybir.EngineType.Activation,
                        mybir.EngineType.DVE,
                        mybir.EngineType.Pool,
                    ),
                ):
                    body()

    if dedup_ldw:
        _dedup_ldweights(nc, mybir)
    nc.compile()
    return nc


def _get_program():
    if "nc" not in _prog_cache:
        _prog_cache["nc"] = _build_program()
    return _prog_cache["nc"]


def _conv_np_dtype(conv_dtype: str | None = None):
    dt = conv_dtype or CONV_DTYPE
    if dt == "bf16":
        import ml_dtypes

        return ml_dtypes.bfloat16
    return {"f32r": np.float32, "fp16": np.float16}[dt]


def _host_prep(x, c, weight, c_weight, bias, conv_dtype: str | None = None):
    # Phase-split padded x: xq[n, ph*64 + i, g, j] = xpad[n, g*64+i, 2j+ph]
    xp = np.zeros((N, C_IN, L + 2 * PAD), np.float32)
    xp[:, :, PAD:PAD + L] = x
    # (N, 4, 64, MPAD, 2) -> (N, 2, 64, 4, MPAD) = [N, 128, G, MPAD]
    xq = np.ascontiguousarray(
        xp.reshape(N, GROUPS, 64, MPAD, 2).transpose(0, 4, 2, 1, 3)
    ).reshape(N, 128, GROUPS, MPAD)

    # Polyphase stationary operands.
    wq = np.zeros((128, GROUPS, NSHIFT, 128), np.float32)
    for g in range(GROUPS):
        wg = weight[g * 64:(g + 1) * 64]          # (64 o, 64 i, K)
        for s in range(NSHIFT):
            wq[0:64, g, s, 0:64] = wg[:, :, 2 * s].T
            if 2 * s + 1 < K:
                wq[64:128, g, s, 0:64] = wg[:, :, 2 * s + 1].T
            if 2 * s - 1 >= 0:
                wq[0:64, g, s, 64:128] = wg[:, :, 2 * s - 1].T
            wq[64:128, g, s, 64:128] = wg[:, :, 2 * s].T

    # Packed consts cc = [cwT2 (G*128) | cT (N, all batches) | biasT2 (G)]
    # on 128 partitions; per-core slicing keeps that core's NB cT columns.
    # cwT2[d, g*128 + 64*delta + o] = c_weight[g*64 + o, d].
    cc = np.zeros((C_DIM, GROUPS * 128 + N + GROUPS), np.float32)
    cw = c_weight.reshape(GROUPS, 64, C_DIM)
    for g in range(GROUPS):
        cc[:, g * 128:g * 128 + 64] = cw[g].T
        cc[:, g * 128 + 64:g * 128 + 128] = cw[g].T
    cb = GROUPS * 128
    cc[:, cb:cb + N] = 0.0  # cT filled per-core in kernel()

    # bias2[(o,d), g] = bias[g*64+o], f32 (tensor_scalar_add needs an f32
    # scalar operand; everything else in cc rides the conv dtype).
    bias2 = np.zeros((128, GROUPS), np.float32)
    b = bias.reshape(GROUPS, 64)
    bias2[0:64] = b.T
    bias2[64:128] = b.T

    npdt = _conv_np_dtype(conv_dtype)
    xq = xq.astype(npdt, copy=False)
    wq = wq.astype(npdt, copy=False)
    cT = np.ascontiguousarray(c.T)  # (128, N)
    return xq, wq, cc, cT, bias2


def _core_in_maps(xq, wq, cc, bias2):
    cb = GROUPS * 128
    npdt = _conv_np_dtype()
    in_maps = []
    for i in range(NCORES):
        cci = np.concatenate(
            [cc[:, :cb], cc[:, cb + i * NB:cb + (i + 1) * NB],
             cc[:, cb + N:cb + N + GROUPS]], axis=1)
        assert cci.shape == (C_DIM, CCW)
        in_maps.append({
            "xq": np.ascontiguousarray(xq[i * NB:(i + 1) * NB]),
            "wq": wq,
            "cc": np.ascontiguousarray(cci.astype(npdt, copy=False)),
            "bias2": bias2,
        })
    return in_maps


def kernel(x, c, weight, c_weight, bias):
    global LAST_RESULT
    from concourse.bass_utils import run_bass_kernel_spmd

    x = np.asarray(x, dtype=np.float32)
    c = np.asarray(c, dtype=np.float32)
    weight = np.asarray(weight, dtype=np.float32)
    c_weight = np.asarray(c_weight, dtype=np.float32)
    bias = np.asarray(bias, dtype=np.float32)

    xq, wq, cc, cT, bias2 = _host_prep(x, c, weight, c_weight, bias)
    cb = GROUPS * 128
    cc[:, cb:cb + N] = cT

    in_maps = _core_in_maps(xq, wq, cc, bias2)

    nc = _get_program()
    res = run_bass_kernel_spmd(nc, in_maps, core_ids=list(range(NCORES)),
                               **RUN_KWARGS)
    LAST_RESULT = res

    # outq per core: [NB, 128, G, M] -> full [N, 128, G, M]
    outq = np.concatenate([r["outq"] for r in res.results], axis=0)
    # y[n, g*64+o, 2m+d] = outq[n, 64d+o, g, m]
    y = np.ascontiguousarray(
        outq.astype(np.float32, copy=False)
        .reshape(N, 2, 64, GROUPS, M)
        .transpose(0, 3, 2, 4, 1)
    ).reshape(N, C_OUT, L)
    return np.ascontiguousarray(y)
